# revision 9
# baseline (speedup 1.0000x reference)
"""Trainium2 Bass kernel for the GNN predictor (2-layer GCN + classifier).

Math (per batch b, with A = where(m>0, m, 0) = m since m is uniform[0,1)):
    deg[c]  = sum_r (m + I)[r, c]                  (includes the +1 self-loop)
    dinv    = 1/sqrt(deg)
    B       = (m + I) * (dinv outer dinv)          [R, R]
    out1    = lrelu(B^T @ X @ W1 + b1)             [R, H]
    out2    = lrelu(B^T @ out1 @ W2 + b2)          [R, H]
    logits  = vec(out2) @ Wc^T + bc                [2]

Transpose-free matmul chain on the PE array (out = lhsT^T @ rhs):
    G1T = matmul(lhsT=X,  rhs=B)       -> [DIN, R]   (= X^T B)
    O1T = matmul(lhsT=W1, rhs=G1T)     -> [H,  R]    (= (B^T X W1)^T), +b1/lrelu on ACT
    M2  = matmul(lhsT=O1T, rhs=W2)     -> [R,  H]    (= out1 @ W2, K=H accumulated)
    O2T = matmul(lhsT=M2, rhs=B)       -> [H,  R]    (+b2/lrelu on ACT)
    cls : 232 accumulating matmuls over 128-wide K chunks of vec(out2),
          rhs = strided columns of the packed O2T region across 64 batches.

Sharding: pure data parallel, 64 batches per core across 8 cores.
All PE matmuls in bf16 (fp32 PSUM accumulation).
"""

import numpy as np
import ml_dtypes

BF16 = ml_dtypes.bfloat16
BZ, R, DIN, H = 512, 116, 116, 256
NCORES = 8
BPC = BZ // NCORES          # 64 batches per core
NB = 4                      # batches packed per group (464 = 4*116 <= 512 psum)
NG = BPC // NB              # 16 groups
HH = H // 2                 # 128, half of hidden dim

_CACHE: dict = {}


def _build_program():
    import concourse.bacc as bacc
    import concourse.mybir as mybir
    import concourse.tile as tile
    from contextlib import ExitStack

    dt = mybir.dt
    f32, b16 = dt.float32, dt.bfloat16
    AF = mybir.ActivationFunctionType
    OP = mybir.AluOpType

    nc = bacc.Bacc("TRN2", target_bir_lowering=False, debug=False,
                   num_devices=NCORES)

    ap_d = nc.dram_tensor("ap", [BPC, R, R], b16, kind="ExternalInput")
    x_d = nc.dram_tensor("x", [BPC, R, DIN], b16, kind="ExternalInput")
    w1_d = nc.dram_tensor("w1", [DIN, H], b16, kind="ExternalInput")
    w2a_d = nc.dram_tensor("w2a", [HH, H], b16, kind="ExternalInput")
    w2b_d = nc.dram_tensor("w2b", [HH, H], b16, kind="ExternalInput")
    wct_d = nc.dram_tensor("wct", [128, R * 4], b16, kind="ExternalInput")
    b1a_d = nc.dram_tensor("b1a", [HH, 1], f32, kind="ExternalInput")
    b1b_d = nc.dram_tensor("b1b", [HH, 1], f32, kind="ExternalInput")
    b2a_d = nc.dram_tensor("b2a", [HH, 1], f32, kind="ExternalInput")
    b2b_d = nc.dram_tensor("b2b", [HH, 1], f32, kind="ExternalInput")
    ones_d = nc.dram_tensor("ones", [R, 1], b16, kind="ExternalInput")
    bcc_d = nc.dram_tensor("bcc", [2, 1], f32, kind="ExternalInput")
    out_d = nc.dram_tensor("out", [2, BPC], f32, kind="ExternalOutput")

    W = NB * R  # 464, packed free width per group

    with tile.TileContext(nc) as tc, ExitStack() as ctx:
        cpool = ctx.enter_context(tc.tile_pool(name="const", bufs=1))
        bigp = ctx.enter_context(tc.tile_pool(name="big", bufs=1))
        inp = ctx.enter_context(tc.tile_pool(name="inp", bufs=3))
        dvp = ctx.enter_context(tc.tile_pool(name="dv", bufs=3))
        btp = ctx.enter_context(tc.tile_pool(name="bt", bufs=2 * NB))
        g1p = ctx.enter_context(tc.tile_pool(name="g1", bufs=2))
        o1p = ctx.enter_context(tc.tile_pool(name="o1", bufs=4))
        m2p = ctx.enter_context(tc.tile_pool(name="m2", bufs=2 * NB))
        # PSUM pools: 3 (early) + 2 (z) + 2 (m2ps) + 1 (cls) = 8 banks
        earlyp = ctx.enter_context(tc.tile_pool(name="early", bufs=3, space="PSUM"))
        zp = ctx.enter_context(tc.tile_pool(name="zp", bufs=2, space="PSUM"))
        m2ps = ctx.enter_context(tc.tile_pool(name="m2ps", bufs=2, space="PSUM"))
        clsp = ctx.enter_context(tc.tile_pool(name="cls", bufs=1, space="PSUM"))

        # --- load constants ---
        w1s = cpool.tile([DIN, H], b16)
        nc.sync.dma_start(w1s[:, :], w1_d.ap())
        w2as = cpool.tile([HH, H], b16)
        nc.sync.dma_start(w2as[:, :], w2a_d.ap())
        w2bs = cpool.tile([HH, H], b16)
        nc.sync.dma_start(w2bs[:, :], w2b_d.ap())
        wcts = cpool.tile([128, R * 4], b16)
        nc.sync.dma_start(wcts[:, :], wct_d.ap())
        b1s = [cpool.tile([HH, 1], f32, name=f"b1s{h}") for h in range(2)]
        nc.sync.dma_start(b1s[0][:, :], b1a_d.ap())
        nc.sync.dma_start(b1s[1][:, :], b1b_d.ap())
        b2s = [cpool.tile([HH, 1], f32, name=f"b2s{h}") for h in range(2)]
        nc.sync.dma_start(b2s[0][:, :], b2a_d.ap())
        nc.sync.dma_start(b2s[1][:, :], b2b_d.ap())
        oness = cpool.tile([R, 1], b16)
        nc.sync.dma_start(oness[:, :], ones_d.ap())
        bccs = cpool.tile([2, 1], f32)
        nc.sync.dma_start(bccs[:, :], bcc_d.ap())
        alpha = cpool.tile([HH, 1], f32)
        nc.vector.memset(alpha[:, :], 0.2)

        # packed O2T region: big[h][p, b*R + c] = out2[b][c, h*128+p]
        big = [bigp.tile([128, BPC * R], b16, name=f"big{h}") for h in range(2)]

        for g in range(NG):
            b0 = NB * g
            # --- load group inputs (one DMA each, b-interleaved layout) ---
            apk = inp.tile([R, W], b16, tag="apk")
            nc.sync.dma_start(
                apk[:, :].rearrange("r (b c) -> r b c", c=R),
                ap_d.ap()[b0:b0 + NB].rearrange("b r c -> r b c"),
            )
            xpk = inp.tile([R, W], b16, tag="xpk")
            nc.sync.dma_start(
                xpk[:, :].rearrange("r (b c) -> r b c", c=DIN),
                x_d.ap()[b0:b0 + NB].rearrange("b r c -> r b c"),
            )

            # --- degree / dinv for the 4 batches at once ---
            csps = earlyp.tile([1, W], f32, tag="early")
            nc.tensor.matmul(csps[:, :], oness[:, :], apk[:, :],
                             start=True, stop=True)
            rcp = dvp.tile([1, W], f32, tag="rcp")
            nc.vector.reciprocal(rcp[:, :], csps[:, :])
            dv = dvp.tile([1, W], b16, tag="dv")
            nc.scalar.activation(dv[:, :], rcp[:, :], AF.Sqrt)

            bts = []
            for j in range(NB):
                sl = slice(R * j, R * (j + 1))
                dps = earlyp.tile([R, R], f32, tag="early")
                nc.tensor.matmul(dps[:, :], dv[:, sl], dv[:, sl],
                                 start=True, stop=True)
                bt = btp.tile([R, R], b16, tag="bt")
                nc.vector.tensor_tensor(bt[:, :], apk[:, sl], dps[:, :], OP.mult)
                bts.append(bt)

            # --- G1T = X^T B, packed into [DIN, W] ---
            g1pk = g1p.tile([DIN, W], b16, tag="g1pk")
            for j in range(NB):
                sl = slice(R * j, R * (j + 1))
                g1ps = earlyp.tile([DIN, R], f32, tag="early")
                nc.tensor.matmul(g1ps[:, :], xpk[:, sl], bts[j][:, :],
                                 start=True, stop=True)
                nc.vector.tensor_copy(g1pk[:, sl], g1ps[:, :])

            # --- O1T = lrelu(W1^T G1T + b1) ---
            o1 = []
            for h in range(2):
                z1 = zp.tile([HH, W], f32, tag="z")
                nc.tensor.matmul(z1[:, :], w1s[:, HH * h:HH * (h + 1)],
                                 g1pk[:, :], start=True, stop=True)
                o1t = o1p.tile([HH, W], b16, tag="o1")
                nc.scalar.activation(o1t[:, :], z1[:, :], AF.Prelu,
                                     bias=b1s[h][:, :], alpha=alpha[:, :])
                o1.append(o1t)

            # --- M2 = out1 @ W2 (K = H accumulated over halves) ---
            m2s = []
            for j in range(NB):
                sl = slice(R * j, R * (j + 1))
                mps = m2ps.tile([R, H], f32, tag="m2ps")
                nc.tensor.matmul(mps[:, :], o1[0][:, sl], w2as[:, :],
                                 start=True, stop=False)
                nc.tensor.matmul(mps[:, :], o1[1][:, sl], w2bs[:, :],
                                 start=False, stop=True)
                m2t = m2p.tile([R, H], b16, tag="m2")
                nc.vector.tensor_copy(m2t[:, :], mps[:, :])
                m2s.append(m2t)

            # --- O2T = lrelu(M2^T B + b2) -> big region ---
            z2 = [zp.tile([HH, W], f32, tag="z", name=f"z2_{g}_{h}") for h in range(2)]
            for j in range(NB):
                sl = slice(R * j, R * (j + 1))
                for h in range(2):
                    nc.tensor.matmul(z2[h][:, sl],
                                     m2s[j][:, HH * h:HH * (h + 1)],
                                     bts[j][:, :], start=True, stop=True)
            for h in range(2):
                nc.scalar.activation(big[h][:, W * g:W * (g + 1)], z2[h][:, :],
                                     AF.Prelu, bias=b2s[h][:, :], alpha=alpha[:, :])

        # --- classifier: logits^T[k, b] accumulated over 232 K-chunks ---
        cls = clsp.tile([2, BPC], f32, tag="cls")
        n_chunks = R * 2
        idx = 0
        for c in range(R):
            for h in range(2):
                rhs = big[h][:, :].rearrange("p (b c) -> p c b", c=R)[:, c, :]
                nc.tensor.matmul(cls[:, :], wcts[:, 4 * c + 2 * h:4 * c + 2 * h + 2],
                                 rhs, start=(idx == 0), stop=(idx == n_chunks - 1))
                idx += 1

        logit = cpool.tile([2, BPC], f32)
        nc.vector.tensor_scalar(logit[:, :], cls[:, :], bccs[:, :], None, OP.add)
        nc.sync.dma_start(out_d.ap(), logit[:, :])

    nc.compile()
    return nc


def _get_program():
    if "nc" not in _CACHE:
        _CACHE["nc"] = _build_program()
    return _CACHE["nc"]


def _prep_inputs(m, node_feature, W1, b1, W2, b2, Wc, bc):
    m = np.asarray(m, np.float32)
    node_feature = np.asarray(node_feature, np.float32)
    W1 = np.asarray(W1, np.float32)
    b1 = np.asarray(b1, np.float32)
    W2 = np.asarray(W2, np.float32)
    b2 = np.asarray(b2, np.float32)
    Wc = np.asarray(Wc, np.float32)
    bc = np.asarray(bc, np.float32)

    Ap = (m + np.eye(R, dtype=np.float32)[None]).astype(BF16)     # [BZ, R, R]
    X = node_feature[:, 0, :].reshape(BZ, R, DIN).astype(BF16)    # [BZ, R, DIN]
    w1h = np.ascontiguousarray(W1.astype(BF16))
    w2a = np.ascontiguousarray(W2[:HH].astype(BF16))
    w2b = np.ascontiguousarray(W2[HH:].astype(BF16))
    # wct[p, c*4 + h*2 + k] = Wc[k, c*256 + h*128 + p]
    wct = np.ascontiguousarray(
        Wc.reshape(2, R, 2, 128).transpose(3, 1, 2, 0).reshape(128, R * 4)
    ).astype(BF16)
    b1a = np.ascontiguousarray(b1[:HH].reshape(HH, 1))
    b1b = np.ascontiguousarray(b1[HH:].reshape(HH, 1))
    b2a = np.ascontiguousarray(b2[:HH].reshape(HH, 1))
    b2b = np.ascontiguousarray(b2[HH:].reshape(HH, 1))
    ones = np.ones((R, 1), BF16)
    bcc = np.ascontiguousarray(bc.reshape(2, 1))

    in_maps = []
    for i in range(NCORES):
        in_maps.append({
            "ap": np.ascontiguousarray(Ap[i * BPC:(i + 1) * BPC]),
            "x": np.ascontiguousarray(X[i * BPC:(i + 1) * BPC]),
            "w1": w1h, "w2a": w2a, "w2b": w2b, "wct": wct,
            "b1a": b1a, "b1b": b1b, "b2a": b2a, "b2b": b2b,
            "ones": ones, "bcc": bcc,
        })
    return in_maps


def kernel(m, node_feature, W1, b1, W2, b2, Wc, bc, _trace=False):
    from concourse.bass_utils import run_bass_kernel_spmd

    nc = _get_program()
    in_maps = _prep_inputs(m, node_feature, W1, b1, W2, b2, Wc, bc)
    res = run_bass_kernel_spmd(nc, in_maps, list(range(NCORES)), trace=_trace)
    _CACHE["last_result"] = res
    outs = [r["out"] for r in res.results]            # [2, BPC] each
    return np.concatenate([o.T for o in outs], axis=0).astype(np.float32)
